# revision 1
# baseline (speedup 1.0000x reference)
"""Trainium2 Bass kernel for a single attention head.

Reference math (per batch b):
    q = emb @ Wq.T + bq ; k = emb @ Wk.T + bk ; v = emb @ Wv.T + bv
    attn = softmax((q @ k.T) / sqrt(768), axis=-1)
    out  = attn @ v

Sharding: pure data-parallel over batch. B=8 batches onto 8 NeuronCores,
one batch per core, no collectives.

Device-side layout strategy (per core):
  - emb arrives pre-transposed from the host as embT (768, 2048) bf16 so the
    E=768 contraction dim sits on SBUF partitions for all three projections.
  - weights arrive as one (768, 320) bf16 block [WqT|WqT | WkT|WkT | WvT]:
    the duplicated Q/K columns produce Q^T / K^T replicated into both
    partition halves (rows 0:64 and 64:128), which lets the score matmuls
    (contraction dim d=64) run pairwise-concurrent via PE row tiling.
  - bk is dropped: softmax over k is invariant to the q.bk + bq.bk terms, so
    (q+bq).(k+bk) and (q+bq).k give identical softmax outputs.
  - scores are computed transposed, S^T[k, q], so exp(S^T) feeds the AV
    matmul directly as the moving operand with k on partitions.
  - no max-subtraction in softmax: scores*scale has std ~0.3, |max| < ~3,
    exp is safe in f32.
  - the softmax denominator Z = sum_k exp rides as a 65th all-ones column of
    V through the same AV matmuls; the final divide folds it out.
"""

import sys

import numpy as np

try:
    import concourse.bass as bass  # noqa: F401
except ImportError:  # pragma: no cover
    sys.path.insert(0, "/opt/trn_rl_repo")

from contextlib import ExitStack

import ml_dtypes

import concourse.bass as bass
import concourse.tile as tile
from concourse import mybir
from concourse.bass_utils import run_bass_kernel_spmd
from concourse.masks import make_identity

S = 2048  # sequence length
E = 768  # embedding dim
D = 64  # inner (head) dim
NCORES = 8
SCALE = float(1.0 / np.sqrt(np.float32(768.0)))

F32 = mybir.dt.float32
BF16 = mybir.dt.bfloat16
AF = mybir.ActivationFunctionType

QB = 512  # q block (one PSUM bank of f32)
NQB = S // QB  # 4 q blocks
NKT = S // 128  # 16 k tiles of 128
NKP = NKT // 2  # 8 k tile pairs


def split_multi_waits(nc: bass.Bass) -> int:
    """This toolchain's walrus encodes at most ONE semaphore wait per
    instruction ("Too many sync wait commands" otherwise). Tile freely emits
    multi-wait instructions, so hoist all but the last wait onto preceding
    same-engine NoOps — sequencer waits gate dispatch, so semantics are
    identical."""
    nsplit = 0
    for f in nc.m.functions:
        for bb in f.blocks:
            out = []
            changed = False
            for inst in bb.instructions:
                si = getattr(inst, "sync_info", None)
                if si is not None and len(si.on_wait) > 1:
                    waits = list(si.on_wait)
                    for w in waits[:-1]:
                        out.append(
                            mybir.InstNoOp(
                                name=nc.get_next_instruction_name(),
                                engine=inst.engine,
                                bass_nofuse=True,
                                sync_info=mybir.SyncInfo(on_wait=[w], on_update=[]),
                            )
                        )
                    inst.sync_info = mybir.SyncInfo(
                        on_wait=[waits[-1]], on_update=list(si.on_update)
                    )
                    changed = True
                    nsplit += 1
                out.append(inst)
            if changed:
                bb.instructions = out
    return nsplit


def build_nc(variant: str = "full", reps: int = 1) -> bass.Bass:
    do_proj = variant in ("full", "proj", "projattn")
    do_attn = variant in ("full", "projattn")
    do_out = variant == "full"
    nc = bass.Bass()

    embT_h = nc.declare_dram_parameter("embT", [E, S], BF16, isOutput=False)
    wts_h = nc.declare_dram_parameter("wts", [E, 320], BF16, isOutput=False)
    bias_h = nc.declare_dram_parameter("biases", [128, 2], F32, isOutput=False)
    out_h = nc.declare_dram_parameter("out", [S, D], F32, isOutput=True)

    with tile.TileContext(nc) as tc, ExitStack() as ctx:
        const = ctx.enter_context(tc.tile_pool(name="const", bufs=1))
        sb = ctx.enter_context(tc.tile_pool(name="sb", bufs=1))

        # ---- constants / small inputs ----
        bias_sb = const.tile([128, 2], F32, tag="bias")
        nc.sync.dma_start(out=bias_sb[:], in_=bias_h[:])
        # weights: one DMA, (768, 320) -> (128, 6, 320)
        wts_all = const.tile([128, 6, 320], BF16, tag="wts")
        nc.sync.dma_start(
            out=wts_all[:], in_=wts_h[:].rearrange("(c p) w -> p c w", p=128)
        )
        ident_bf = const.tile([128, 128], BF16, tag="idbf")
        make_identity(nc, ident_bf[:])
        ident_f32 = const.tile([128, 128], F32, tag="idf32")
        make_identity(nc, ident_f32[:])

        # warm the ACT exp table set while DMAs run
        warm = const.tile([128, 8], F32, tag="warm")
        nc.gpsimd.memset(warm[:], 0.0)
        nc.scalar.activation(warm[:], warm[:], AF.Exp)

        # ---- persistent SBUF ----
        qt_sb = sb.tile([128, S], BF16, tag="qt")
        kt_sb = sb.tile([128, S], BF16, tag="kt")
        vt_sb = sb.tile([64, S], BF16, tag="vt")
        # V' tiles: (k-tile, 65) with col 64 == 1.0 (softmax denominator)
        vv_sb = sb.tile([128, NKT, D + 1], BF16, tag="vv")
        nc.gpsimd.memset(vv_sb[:, :, D : D + 1], 1.0)
        out_sb = sb.tile([128, NKT, D], F32, tag="outsb")

        embT_sb = [[None] * NQB for _ in range(6)]

        def dma_embT_chunk(n):
            # column-major arrival: projection chunk n contracts over all six
            # e-chunks of q-columns [n*512,(n+1)*512); chunk n ready ~750KB in
            for c in range(6):
                t = sb.tile([128, QB], BF16, tag=f"embT{c}_{n}")
                nc.sync.dma_start(
                    out=t[:],
                    in_=embT_h[c * 128 : (c + 1) * 128, n * QB : (n + 1) * QB],
                )
                embT_sb[c][n] = t

        with (
            tc.tile_pool(name="psA", bufs=1, space="PSUM") as psA,
            tc.tile_pool(name="psT", bufs=1, space="PSUM") as psT,
            tc.tile_pool(name="psO", bufs=2, space="PSUM") as psO,
            tc.tile_pool(name="psS", bufs=2, space="PSUM") as psS,
            tc.tile_pool(name="ptp", bufs=3) as ptp,
            tc.tile_pool(name="osb", bufs=2) as osb,
            tc.tile_pool(name="rcp", bufs=4) as rcp,
        ):
            oacc_tiles = {}

            def proj_chunk(name, n):
                col0 = {"q": 0, "k": 128, "v": 256}[name]
                m = 128 if name != "v" else 64
                qs = slice(n * QB, (n + 1) * QB)
                ps = psA.tile([128, QB], F32, tag="proj")
                for c in range(6):
                    nc.tensor.matmul(
                        ps[0:m, :],
                        lhsT=wts_all[:, c, col0 : col0 + m],
                        rhs=embT_sb[c][n][:, :],
                        start=(c == 0),
                        stop=(c == 5),
                    )
                if name == "q":
                    nc.vector.tensor_scalar_add(
                        qt_sb[:, qs], ps[:, :], bias_sb[:, 0:1]
                    )
                elif name == "k":
                    nc.vector.tensor_copy(out=kt_sb[:, qs], in_=ps[:, :])
                else:
                    nc.vector.tensor_scalar_add(
                        vt_sb[0:64, qs], ps[0:64, :], bias_sb[0:64, 1:2]
                    )
                    # V^T chunk -> 4 V' tiles (128, 64) via PE transpose
                    vtp = psT.tile([128, 256], BF16, tag="vtp")
                    for jj in range(4):
                        j = 4 * n + jj
                        nc.tensor.transpose(
                            vtp[:, jj * 64 : (jj + 1) * 64],
                            vt_sb[0:64, j * 128 : (j + 1) * 128],
                            ident_bf[0:64, 0:64],
                        )
                    nc.vector.tensor_copy(
                        out=vv_sb[:, 4 * n : 4 * n + 4, 0:D],
                        in_=vtp[:].rearrange("p (j d) -> p j d", j=4),
                    )

            def attn_pair(n, j):
                qs = slice(n * QB, (n + 1) * QB)
                if j == 0:
                    oacc_tiles[n] = psO.tile([128, QB], F32, tag="oacc", name=f"oacc{rep}_{n}")
                oacc = oacc_tiles[n]
                sc = psS.tile([128, 1024], F32, tag="sc")
                # S^T tiles for k-tiles 2j (partitions 0:64) and 2j+1
                # (partitions 64:128) — concurrent via PE row tiling.
                nc.tensor.matmul(
                    sc[:, 0:512],
                    lhsT=kt_sb[0:64, (2 * j) * 128 : (2 * j + 1) * 128],
                    rhs=qt_sb[0:64, qs],
                    start=True,
                    stop=True,
                )
                nc.tensor.matmul(
                    sc[:, 512:1024],
                    lhsT=kt_sb[64:128, (2 * j + 1) * 128 : (2 * j + 2) * 128],
                    rhs=qt_sb[64:128, qs],
                    start=True,
                    stop=True,
                )
                pt = ptp.tile([128, 1024], BF16, tag="pt")
                nc.scalar.activation(pt[:], sc[:], AF.Exp, scale=SCALE)
                nc.tensor.matmul(
                    oacc[0 : D + 1, :],
                    lhsT=vv_sb[:, 2 * j, :],
                    rhs=pt[:, 0:512],
                    start=(j == 0),
                    stop=False,
                    skip_group_check=True,
                )
                nc.tensor.matmul(
                    oacc[0 : D + 1, :],
                    lhsT=vv_sb[:, 2 * j + 1, :],
                    rhs=pt[:, 512:1024],
                    start=False,
                    stop=(j == NKP - 1),
                    skip_group_check=True,
                )

            def out_block(n):
                qs = slice(n * QB, (n + 1) * QB)
                oacc = oacc_tiles[n]
                # evacuate O'^T (65, 512), transpose 128-col chunks, divide
                o_sb = osb.tile([65, QB], F32, tag="osb")
                nc.vector.tensor_copy(out=o_sb[:], in_=oacc[0 : D + 1, :])
                for t in range(4):
                    qt_idx = n * 4 + t
                    otp = psT.tile([128, 128], F32, tag="vtp")
                    nc.tensor.transpose(
                        otp[:, 0 : D + 1],
                        o_sb[:, t * 128 : (t + 1) * 128],
                        ident_f32[0:65, 0:65],
                    )
                    rc = rcp.tile([128, 1], F32, tag="rc")
                    nc.vector.reciprocal(rc[:], otp[:, D : D + 1])
                    nc.vector.tensor_scalar_mul(
                        out_sb[:, qt_idx, :], otp[:, 0:D], rc[:, 0:1]
                    )
                nc.sync.dma_start(
                    out=out_h[qs, :].rearrange("(t p) i -> p t i", p=128),
                    in_=out_sb[:, n * 4 : (n + 1) * 4, :],
                )

            # ---- software-pipelined emission: projections paced by the
            # column-major embT arrival, attention pairs for q-blocks 0/1
            # interleaved as their K/V chunks complete, q-blocks 2/3 after.
            # reps > 1 repeats the whole computation for benchmarking.
            for rep in range(reps):
              for n in range(NQB):
                dma_embT_chunk(n)
              if rep == 0 and do_attn:
                  # PE warm-up during the DMA lead-in: dummy matmuls on the
                  # identity tile ramp the HAM clock gate (1.2 -> 2.4 GHz)
                  # before the first real projection matmul.
                  wmm = psS.tile([128, 1024], F32, tag="sc", name="warmmm")
                  for i in range(12):
                      nc.tensor.matmul(
                          wmm[:, (i % 2) * 512 : (i % 2) * 512 + 128],
                          lhsT=ident_bf[:, 0:128],
                          rhs=ident_bf[:, 0:128],
                          start=True,
                          stop=True,
                      )
              if do_proj:
                  proj_chunk("q", 0)
                  proj_chunk("k", 0)
                  proj_chunk("v", 0)
              if do_attn:
                  attn_pair(0, 0)
                  attn_pair(0, 1)
              if do_proj:
                  proj_chunk("q", 1)
              if do_attn:
                  attn_pair(1, 0)
                  attn_pair(1, 1)
              if do_proj:
                  proj_chunk("k", 1)
                  proj_chunk("v", 1)
              if do_attn:
                  for n in (0, 1):
                      attn_pair(n, 2)
                      attn_pair(n, 3)
              if do_proj:
                  proj_chunk("k", 2)
                  proj_chunk("v", 2)
              if do_attn:
                  for n in (0, 1):
                      attn_pair(n, 4)
                      attn_pair(n, 5)
              if do_proj:
                  proj_chunk("k", 3)
                  proj_chunk("v", 3)
                  proj_chunk("q", 2)
              if do_attn:
                  for n in (0, 1):
                      attn_pair(n, 6)
                      attn_pair(n, 7)
                  if do_out:
                      out_block(0)
                      out_block(1)
              if do_proj:
                  proj_chunk("q", 3)
              if do_attn:
                  for j in range(NKP):
                      attn_pair(2, j)
                  if do_out:
                      out_block(2)
                  for j in range(NKP):
                      attn_pair(3, j)
                  if do_out:
                      out_block(3)
            if not do_out:
                nc.gpsimd.memset(out_sb[:, 0:1, :], 0.0)
                nc.sync.dma_start(
                    out=out_h[:].rearrange("(t p) i -> p t i", p=128),
                    in_=out_sb[:],
                )

    split_multi_waits(nc)
    return nc


_NC_CACHE = None


def _get_nc():
    global _NC_CACHE
    if _NC_CACHE is None:
        _NC_CACHE = build_nc()
    return _NC_CACHE


def make_in_maps(emb_input, Wq, bq, Wk, bk, Wv, bv):
    bf16 = ml_dtypes.bfloat16
    WqT = np.ascontiguousarray(Wq.T).astype(bf16)  # (768, 64)
    WkT = np.ascontiguousarray(Wk.T).astype(bf16)
    WvT = np.ascontiguousarray(Wv.T).astype(bf16)
    wts = np.ascontiguousarray(
        np.concatenate([WqT, WqT, WkT, WkT, WvT], axis=1)
    )  # (768, 320)
    biases = np.zeros((128, 2), np.float32)
    biases[0:64, 0] = bq
    biases[64:128, 0] = bq
    biases[0:64, 1] = bv
    in_maps = []
    for i in range(NCORES):
        embT = np.ascontiguousarray(emb_input[i].T).astype(bf16)  # (768, 2048)
        in_maps.append({"embT": embT, "wts": wts, "biases": biases})
    return in_maps


def run(emb_input, Wq, bq, Wk, bk, Wv, bv, trace=False):
    nc = _get_nc()
    in_maps = make_in_maps(emb_input, Wq, bq, Wk, bk, Wv, bv)
    res = run_bass_kernel_spmd(nc, in_maps, core_ids=list(range(NCORES)), trace=trace)
    out = np.stack([res.results[i]["out"] for i in range(NCORES)], axis=0)
    return out.astype(np.float32), res


def kernel(emb_input, Wq, bq, Wk, bk, Wv, bv):
    out, _ = run(emb_input, Wq, bq, Wk, bk, Wv, bv, trace=False)
    return out



# revision 5
# speedup vs baseline: 1.0030x; 1.0030x over previous
"""Trainium2 Bass kernel for a single attention head.

Reference math (per batch b):
    q = emb @ Wq.T + bq ; k = emb @ Wk.T + bk ; v = emb @ Wv.T + bv
    attn = softmax((q @ k.T) / sqrt(768), axis=-1)
    out  = attn @ v

Sharding: pure data-parallel over batch. B=8 batches onto 8 NeuronCores,
one batch per core, no collectives.

Device-side layout strategy (per core):
  - emb arrives pre-transposed from the host as embT (768, 2048) bf16, one
    DMA per 512-column chunk ([128, 6, 512] tiles), so the E=768 contraction
    dim sits on SBUF partitions and HWDGE dispatch stays off the critical
    path.
  - Q and K projections are FUSED: one stationary [WqT|WkT] (768, 128) block
    computes Q^T (PSUM rows 0:64) and K^T (rows 64:128) in a single moving
    pass over each embT chunk.
  - bk is dropped: softmax over k is invariant to it.
  - Q^T/K^T are evacuated to fp8e4m3 in the DoubleRow-folded layout
    [32, 2, S] (plane j holds head dims j*32:(j+1)*32) via cross-
    partition-base DVE/Pool copies; the score matmuls then run in fp8
    DoubleRow perf mode at 0.5 cycles/row (2x bf16). Adds ~1% rel err
    (measured 1.34e-2 total vs the 2e-2 gate).
  - scores are computed transposed, S^T[k, q]; exp(S^T) (ACT, scale folded
    in) feeds the AV stage as bf16. No max-subtraction: scores*scale has
    std ~0.3, exp is safe in f32.
  - AV runs FLIPPED: P^T 128x128 slices are the stationary operand and
    V' (64 cols of V + an all-ones column for the softmax denominator Z)
    is the 65-column moving operand, accumulated over the 16 k-tiles.
    The output lands q-on-partitions, so no output transposes are needed;
    the final divide folds Z out directly from PSUM.
"""

import sys

import numpy as np

try:
    import concourse.bass as bass  # noqa: F401
except ImportError:  # pragma: no cover
    sys.path.insert(0, "/opt/trn_rl_repo")

from contextlib import ExitStack

import ml_dtypes

import concourse.bass as bass
import concourse.tile as tile
from concourse import mybir
from concourse.bass_utils import run_bass_kernel_spmd
from concourse.masks import make_identity

S = 2048  # sequence length
E = 768  # embedding dim
D = 64  # inner (head) dim
NCORES = 8
SCALE = float(1.0 / np.sqrt(np.float32(768.0)))

F32 = mybir.dt.float32
BF16 = mybir.dt.bfloat16
F8 = mybir.dt.float8e4
AF = mybir.ActivationFunctionType
DR = mybir.MatmulPerfMode.DoubleRow

QB = 512  # q block (one PSUM bank of f32)
NQB = S // QB  # 4 q blocks
NKT = S // 128  # 16 k tiles of 128
NKP = NKT // 2  # 8 k tile pairs


def split_multi_waits(nc: bass.Bass) -> int:
    """This toolchain's walrus encodes at most ONE semaphore wait per
    instruction ("Too many sync wait commands" otherwise). Tile freely emits
    multi-wait instructions, so hoist all but the last wait onto preceding
    same-engine NoOps — sequencer waits gate dispatch, so semantics are
    identical."""
    nsplit = 0
    for f in nc.m.functions:
        for bb in f.blocks:
            out = []
            changed = False
            for inst in bb.instructions:
                si = getattr(inst, "sync_info", None)
                if si is not None and len(si.on_wait) > 1:
                    waits = list(si.on_wait)
                    for w in waits[:-1]:
                        out.append(
                            mybir.InstNoOp(
                                name=nc.get_next_instruction_name(),
                                engine=inst.engine,
                                bass_nofuse=True,
                                sync_info=mybir.SyncInfo(on_wait=[w], on_update=[]),
                            )
                        )
                    inst.sync_info = mybir.SyncInfo(
                        on_wait=[waits[-1]], on_update=list(si.on_update)
                    )
                    changed = True
                    nsplit += 1
                out.append(inst)
            if changed:
                bb.instructions = out
    return nsplit


def build_nc(variant: str = "full", reps: int = 1) -> bass.Bass:
    do_proj = variant in ("full", "proj", "projattn")
    do_attn = variant in ("full", "projattn")
    do_out = variant == "full"
    nc = bass.Bass()

    embT_h = nc.declare_dram_parameter("embT", [E, S], BF16, isOutput=False)
    wts_h = nc.declare_dram_parameter("wts", [E, 192], BF16, isOutput=False)
    bias_h = nc.declare_dram_parameter("biases", [128, 2], F32, isOutput=False)
    out_h = nc.declare_dram_parameter("out", [S, D], F32, isOutput=True)

    with tile.TileContext(nc) as tc, ExitStack() as ctx:
        const = ctx.enter_context(tc.tile_pool(name="const", bufs=1))
        sb = ctx.enter_context(tc.tile_pool(name="sb", bufs=1))

        # ---- constants / small inputs ----
        bias_sb = const.tile([128, 2], F32, tag="bias")
        nc.sync.dma_start(out=bias_sb[:], in_=bias_h[:])
        # weights: one DMA, (768, 192) -> (128, 6, 192): [WqT|WkT|WvT]
        wts_all = const.tile([128, 6, 192], BF16, tag="wts")
        nc.sync.dma_start(
            out=wts_all[:], in_=wts_h[:].rearrange("(c p) w -> p c w", p=128)
        )
        ident_bf = const.tile([128, 128], BF16, tag="idbf")
        make_identity(nc, ident_bf[:])

        # warm the ACT exp table set while DMAs run
        warm = const.tile([128, 8], F32, tag="warm")
        nc.gpsimd.memset(warm[:], 0.0)
        nc.scalar.activation(warm[:], warm[:], AF.Exp)

        # ---- persistent SBUF ----
        # Q^T / K^T in fp8 DoubleRow-folded layout: [32, 2, S], plane j holds
        # head dims j*32:(j+1)*32.
        qf_sb = sb.tile([32, 2, S], F8, tag="qf")
        kf_sb = sb.tile([32, 2, S], F8, tag="kf")
        vt_sb = sb.tile([64, S], BF16, tag="vt")
        # V' tiles: (k-tile, 65) with col 64 == 1.0 (softmax denominator)
        vv_sb = sb.tile([128, NKT, D + 1], BF16, tag="vv")
        nc.gpsimd.memset(vv_sb[:, :, D : D + 1], 1.0)
        out_sb = sb.tile([128, NKT, D], F32, tag="outsb")

        embT_sb = [None] * NQB

        def dma_embT_chunk(n):
            # one DMA per 512-col chunk: (768, 512) -> (128, 6, 512)
            t = sb.tile([128, 6, QB], BF16, tag=f"embT_{n}")
            nc.sync.dma_start(
                out=t[:],
                in_=embT_h[:, n * QB : (n + 1) * QB].rearrange(
                    "(c p) w -> p c w", p=128
                ),
            )
            embT_sb[n] = t

        with (
            tc.tile_pool(name="psA", bufs=1, space="PSUM") as psA,
            tc.tile_pool(name="psT", bufs=1, space="PSUM") as psT,
            tc.tile_pool(name="psO", bufs=2, space="PSUM") as psO,
            tc.tile_pool(name="psS", bufs=2, space="PSUM") as psS,
            tc.tile_pool(name="ptp", bufs=3) as ptp,
            tc.tile_pool(name="rcp", bufs=4) as rcp,
        ):
            oacc_tiles = {}

            def proj_qk_chunk(n):
                qs = slice(n * QB, (n + 1) * QB)
                ps = psA.tile([128, QB], F32, tag="proj")
                for c in range(6):
                    nc.tensor.matmul(
                        ps[:, :],
                        lhsT=wts_all[:, c, 0:128],
                        rhs=embT_sb[n][:, c, :],
                        start=(c == 0),
                        stop=(c == 5),
                    )
                # evacuate + fold to fp8 DoubleRow layout (cross-base copies)
                nc.vector.tensor_scalar_add(
                    qf_sb[:, 0, qs], ps[0:32, :], bias_sb[0:32, 0:1]
                )
                nc.vector.tensor_scalar_add(
                    qf_sb[:, 1, qs], ps[32:64, :], bias_sb[32:64, 0:1]
                )
                nc.vector.tensor_copy(out=kf_sb[:, 0, qs], in_=ps[64:96, :])
                nc.vector.tensor_copy(out=kf_sb[:, 1, qs], in_=ps[96:128, :])

            def proj_v_chunk(n):
                qs = slice(n * QB, (n + 1) * QB)
                ps = psA.tile([128, QB], F32, tag="proj")
                for c in range(6):
                    nc.tensor.matmul(
                        ps[0:64, :],
                        lhsT=wts_all[:, c, 128:192],
                        rhs=embT_sb[n][:, c, :],
                        start=(c == 0),
                        stop=(c == 5),
                    )
                nc.vector.tensor_scalar_add(
                    vt_sb[0:64, qs], ps[0:64, :], bias_sb[0:64, 1:2]
                )
                # V^T chunk -> 4 V' tiles (128, 64) via PE transpose
                vtp = psT.tile([128, 256], BF16, tag="vtp")
                for jj in range(4):
                    j = 4 * n + jj
                    nc.tensor.transpose(
                        vtp[:, jj * 64 : (jj + 1) * 64],
                        vt_sb[0:64, j * 128 : (j + 1) * 128],
                        ident_bf[0:64, 0:64],
                    )
                nc.vector.tensor_copy(
                    out=vv_sb[:, 4 * n : 4 * n + 4, 0:D],
                    in_=vtp[:].rearrange("p (j d) -> p j d", j=4),
                )

            def attn_pair(n, j):
                qs = slice(n * QB, (n + 1) * QB)
                if j == 0:
                    oacc_tiles[n] = psO.tile(
                        [128, 4, D + 1], F32, tag="oacc", name=f"oacc{rep}_{n}"
                    )
                oacc = oacc_tiles[n]
                sc = psS.tile([128, 1024], F32, tag="sc")
                # S^T tiles for k-tiles 2j and 2j+1, fp8 DoubleRow (0.5 cyc/row)
                for h in range(2):
                    kt = 2 * j + h
                    nc.tensor.matmul(
                        sc[:, h * 512 : (h + 1) * 512],
                        lhsT=kf_sb[:, :, kt * 128 : (kt + 1) * 128],
                        rhs=qf_sb[:, :, qs],
                        start=True,
                        stop=True,
                        perf_mode=DR,
                    )
                pt = ptp.tile([128, 1024], BF16, tag="pt")
                nc.scalar.activation(pt[:], sc[:], AF.Exp, scale=SCALE)
                # flipped AV: P^T 128x128 slices stationary, V' 65-col moving
                for h in range(2):
                    kt = 2 * j + h
                    for sub in range(4):
                        nc.tensor.matmul(
                            oacc[:, sub, :],
                            lhsT=pt[:, h * 512 + sub * 128 : h * 512 + (sub + 1) * 128],
                            rhs=vv_sb[:, kt, :],
                            start=(j == 0 and h == 0),
                            stop=(j == NKP - 1 and h == 1),
                            skip_group_check=True,
                        )

            def out_block(n):
                oacc = oacc_tiles[n]
                for sub in range(4):
                    t = n * 4 + sub
                    rc = rcp.tile([128, 1], F32, tag="rc")
                    nc.vector.reciprocal(rc[:], oacc[:, sub, D : D + 1])
                    nc.vector.tensor_scalar_mul(
                        out_sb[:, t, :], oacc[:, sub, 0:D], rc[:, 0:1]
                    )
                qs = slice(n * QB, (n + 1) * QB)
                nc.sync.dma_start(
                    out=out_h[qs, :].rearrange("(t p) i -> p t i", p=128),
                    in_=out_sb[:, n * 4 : (n + 1) * 4, :],
                )

            # ---- software-pipelined emission, paced by chunk DMA arrival.
            # reps > 1 repeats the whole computation for benchmarking.
            for rep in range(reps):
                for n in range(NQB):
                    dma_embT_chunk(n)
                if rep == 0 and do_attn:
                    # PE warm-up during the DMA lead-in: dummy matmuls ramp the
                    # HAM clock gate (1.2 -> 2.4 GHz) before the first real
                    # projection matmul.
                    wmm = psS.tile([128, 1024], F32, tag="sc", name="warmmm")
                    for i in range(12):
                        nc.tensor.matmul(
                            wmm[:, (i % 2) * 512 : (i % 2) * 512 + 128],
                            lhsT=ident_bf[:, 0:128],
                            rhs=ident_bf[:, 0:128],
                            start=True,
                            stop=True,
                        )
                if do_proj:
                    proj_qk_chunk(0)
                    proj_v_chunk(0)
                if do_attn:
                    attn_pair(0, 0)
                    attn_pair(0, 1)
                if do_proj:
                    proj_qk_chunk(1)
                if do_attn:
                    attn_pair(1, 0)
                    attn_pair(1, 1)
                if do_proj:
                    proj_v_chunk(1)
                if do_attn:
                    for n in (0, 1):
                        attn_pair(n, 2)
                        attn_pair(n, 3)
                if do_proj:
                    proj_qk_chunk(2)
                    proj_v_chunk(2)
                if do_attn:
                    for n in (0, 1):
                        attn_pair(n, 4)
                        attn_pair(n, 5)
                if do_proj:
                    proj_qk_chunk(3)
                    proj_v_chunk(3)
                if do_attn:
                    for n in (0, 1):
                        attn_pair(n, 6)
                        attn_pair(n, 7)
                    if do_out:
                        out_block(0)
                        out_block(1)
                    for j in range(NKP):
                        attn_pair(2, j)
                    if do_out:
                        out_block(2)
                    for j in range(NKP):
                        attn_pair(3, j)
                    if do_out:
                        out_block(3)
            if not do_out:
                nc.gpsimd.memset(out_sb[:, 0:1, :], 0.0)
                nc.sync.dma_start(
                    out=out_h[:].rearrange("(t p) i -> p t i", p=128),
                    in_=out_sb[:],
                )

    split_multi_waits(nc)
    return nc


_NC_CACHE = None


def _get_nc():
    global _NC_CACHE
    if _NC_CACHE is None:
        _NC_CACHE = build_nc()
    return _NC_CACHE


def make_in_maps(emb_input, Wq, bq, Wk, bk, Wv, bv):
    bf16 = ml_dtypes.bfloat16
    WqT = np.ascontiguousarray(Wq.T).astype(bf16)  # (768, 64)
    WkT = np.ascontiguousarray(Wk.T).astype(bf16)
    WvT = np.ascontiguousarray(Wv.T).astype(bf16)
    wts = np.ascontiguousarray(
        np.concatenate([WqT, WkT, WvT], axis=1)
    )  # (768, 192)
    biases = np.zeros((128, 2), np.float32)
    biases[0:64, 0] = bq
    biases[0:64, 1] = bv
    in_maps = []
    for i in range(NCORES):
        embT = np.ascontiguousarray(emb_input[i].T).astype(bf16)  # (768, 2048)
        in_maps.append({"embT": embT, "wts": wts, "biases": biases})
    return in_maps


def run(emb_input, Wq, bq, Wk, bk, Wv, bv, trace=False):
    nc = _get_nc()
    in_maps = make_in_maps(emb_input, Wq, bq, Wk, bk, Wv, bv)
    res = run_bass_kernel_spmd(nc, in_maps, core_ids=list(range(NCORES)), trace=trace)
    out = np.stack([res.results[i]["out"] for i in range(NCORES)], axis=0)
    return out.astype(np.float32), res


def kernel(emb_input, Wq, bq, Wk, bk, Wv, bv):
    out, _ = run(emb_input, Wq, bq, Wk, bk, Wv, bv, trace=False)
    return out
